# revision 6
# baseline (speedup 1.0000x reference)
"""GroupedQueryAttention forward, 8-way tensor-parallel over heads on TRN2.

Problem: B=2, T=2048, D=4096, H=32 q-heads, KV=8 kv-heads, HD=128, causal,
interleaved RoPE, softmax, o-proj.

Sharding (8 NeuronCores):
  - core c owns q-heads 4c..4c+3 (Wq rows c*512..) and kv-head c (Wk/Wv rows
    c*128..). x and Wo are replicated.
  - Each core computes q/k/v proj + RoPE + causal attention for its heads over
    ALL tokens, writes attn-out (token-major, bf16) into an AllToAll buffer
    sharded by token-block.
  - AllToAll redistributes: core c ends with the full 4096-dim attn-out for
    tokens c*512..(c+1)*512, computes the o-proj for those tokens, and writes
    its [512, 4096] slice of the output. Host concatenates the 8 slices.

All matmuls run in bf16 (fp32 PSUM accumulation). fp32 operands are cast to
bf16 on-chip; K-first (transposed) operand layouts are produced with the
HWDGE xbar DMA-transpose (bf16-only), never with AP-rearrange DMAs.
"""

import math
import os

import numpy as np

import concourse.bass as bass
import concourse.mybir as mybir
import concourse.tile as tile
from concourse import bacc
from concourse.bass_utils import run_bass_kernel_spmd
from concourse.masks import make_causal_mask

F32 = mybir.dt.float32
BF16 = mybir.dt.bfloat16
AF = mybir.ActivationFunctionType
AX = mybir.AxisListType

B, T, D = 2, 2048, 4096
H, KV, HD = 32, 8, 128
NCORES = 8
BT = B * T                  # 4096 flattened tokens
HL = H // NCORES            # 4 q heads per core
OL = HL * HD                # 512 local o-dim
TT = 128                    # token tile
NT = BT // TT               # 32 token tiles
NTB = T // TT               # 16 token tiles per batch
ND = D // TT                # 32 d(=4096) k-tiles
TOK_SLICE = BT // NCORES    # 512 tokens owned per core for o-proj
SCALE = 1.0 / math.sqrt(HD)
MASK_VAL = -1e9


def _build():
    nc = bacc.Bacc(
        "TRN2",
        target_bir_lowering=False,
        debug=False,
        enable_asserts=False,
        num_devices=NCORES,
    )
    x = nc.dram_tensor("x", [BT, D], F32, kind="ExternalInput").ap()
    freqs = nc.dram_tensor("freqs", [T, HD // 2, 2], F32, kind="ExternalInput").ap()
    wq = nc.dram_tensor("wq", [OL, D], F32, kind="ExternalInput").ap()
    wk = nc.dram_tensor("wk", [HD, D], F32, kind="ExternalInput").ap()
    wv = nc.dram_tensor("wv", [HD, D], F32, kind="ExternalInput").ap()
    wo = nc.dram_tensor("wo", [D, D], F32, kind="ExternalInput").ap()
    out = nc.dram_tensor("out", [TOK_SLICE, D], F32, kind="ExternalOutput").ap()

    dbg = None
    if os.environ.get("KERNEL_DEBUG") == "1":
        dbg = {
            "d_qro": nc.dram_tensor("d_qro", [128, OL], F32, kind="ExternalOutput").ap(),
            "d_kro": nc.dram_tensor("d_kro", [128, HD], F32, kind="ExternalOutput").ap(),
            "d_v": nc.dram_tensor("d_v", [128, HD], F32, kind="ExternalOutput").ap(),
            "d_qT": nc.dram_tensor("d_qT", [128, 128], F32, kind="ExternalOutput").ap(),
            "d_p": nc.dram_tensor("d_p", [128, 512], F32, kind="ExternalOutput").ap(),
            "d_sums": nc.dram_tensor("d_sums", [128, 8], F32, kind="ExternalOutput").ap(),
            "d_attn": nc.dram_tensor("d_attn", [128, HD], F32, kind="ExternalOutput").ap(),
        }

    with tile.TileContext(nc) as tc:
        _body(tc, nc, x, freqs, wq, wk, wv, wo, out, dbg)
    nc.compile()
    return nc


def _body(tc, nc, x, freqs, wq, wk, wv, wo, out, dbg=None):
    # ---- persistent SBUF (lives across phases 1+2) ----
    with tc.tile_pool(name="persist", bufs=1) as persist, \
         tc.tile_pool(name="dram", bufs=1, space="DRAM") as dram:
        # q^T per head: [hd=128, 4 heads * 4096 tokens] bf16
        qT_all = persist.tile([128, HL * BT], BF16)
        # k^T: [hd=128, 4096 tokens] bf16
        kT_all = persist.tile([128, BT], BF16)
        # v token-major: [token-part, 32 tiles * 128 hd] bf16
        v_all = persist.tile([128, NT * HD], BF16)
        # causal mask for diagonal 128x128 tile (fp32)
        maskt = persist.tile([128, 128], F32)
        make_causal_mask(nc, maskt[:, :], mask_val=MASK_VAL)
        # cos/sin for 16 token tiles (per batch): [128, 16*64] fp32
        cos_all = persist.tile([128, NTB * 64], F32)
        sin_all = persist.tile([128, NTB * 64], F32)
        for tt in range(NTB):
            nc.sync.dma_start(
                out=cos_all[:, tt * 64:(tt + 1) * 64],
                in_=freqs[tt * 128:(tt + 1) * 128, :, 0:1].opt(),
            )
            nc.sync.dma_start(
                out=sin_all[:, tt * 64:(tt + 1) * 64],
                in_=freqs[tt * 128:(tt + 1) * 128, :, 1:2].opt(),
            )

        # AllToAll buffers (bf16): shard s = token rows s*512..(s+1)*512
        a2a_in = dram.tile([BT, OL], BF16)
        a2a_out = dram.tile([BT, OL], BF16)

        # ================= Phase 1: qkv proj + rope =================
        with tc.tile_pool(name="p1", bufs=2) as p1, \
             tc.tile_pool(name="p1w", bufs=1) as p1w, \
             tc.tile_pool(name="p1ps", bufs=2, space="PSUM") as p1ps:
            # --- weights: load fp32, cast, xbar-transpose to [d, o] ---
            wqT = p1w.tile([128, ND * OL], BF16)   # slice dt: [:, dt*512 : +512]
            wkT = p1w.tile([128, ND * HD], BF16)   # slice dt: [:, dt*128 : +128]
            wvT = p1w.tile([128, ND * HD], BF16)
            for ob in range(HL):  # 4 row-blocks of wq
                wtmp = p1.tile([128, D], F32, tag="ftmp")
                nc.sync.dma_start(out=wtmp[:, :], in_=wq[ob * 128:(ob + 1) * 128, :])
                wbf = p1.tile([128, D], BF16, tag="btmp")
                nc.scalar.copy(wbf[:, :], wtmp[:, :])
                for dt in range(ND):
                    nc.sync.dma_start(
                        out=wqT[:, dt * OL + ob * 128: dt * OL + (ob + 1) * 128],
                        in_=wbf[:, dt * 128:(dt + 1) * 128],
                        transpose=True,
                    )
            for w_in, w_t in ((wk, wkT), (wv, wvT)):
                wtmp = p1.tile([128, D], F32, tag="ftmp")
                nc.sync.dma_start(out=wtmp[:, :], in_=w_in[:, :])
                wbf = p1.tile([128, D], BF16, tag="btmp")
                nc.scalar.copy(wbf[:, :], wtmp[:, :])
                for dt in range(ND):
                    nc.sync.dma_start(
                        out=w_t[:, dt * HD:(dt + 1) * HD],
                        in_=wbf[:, dt * 128:(dt + 1) * 128],
                        transpose=True,
                    )

            for tt in range(NT):
                tb = tt % NTB  # tile index within batch (for rope)
                # x tile: load fp32, cast, xbar-transpose to xT [d, t]
                xtmp = p1.tile([128, D], F32, tag="ftmp")
                nc.sync.dma_start(out=xtmp[:, :], in_=x[tt * 128:(tt + 1) * 128, :])
                xbf = p1.tile([128, D], BF16, tag="btmp")
                nc.scalar.copy(xbf[:, :], xtmp[:, :])
                xT = p1.tile([128, D], BF16, tag="xT")  # col block dt = [d-part, t]
                for dt in range(ND):
                    nc.sync.dma_start(
                        out=xT[:, dt * 128:(dt + 1) * 128],
                        in_=xbf[:, dt * 128:(dt + 1) * 128],
                        transpose=True,
                    )
                # projections: q [t,512], k [t,128], v [t,128]
                ps_q = p1ps.tile([128, OL], F32, tag="ps_q")
                ps_k = p1ps.tile([128, HD], F32, tag="ps_k")
                ps_v = p1ps.tile([128, HD], F32, tag="ps_v")
                for dt in range(ND):
                    lhs = xT[:, dt * 128:(dt + 1) * 128]
                    nc.tensor.matmul(
                        ps_q[:, :], lhs, wqT[:, dt * OL:(dt + 1) * OL],
                        start=(dt == 0), stop=(dt == ND - 1),
                    )
                    nc.tensor.matmul(
                        ps_k[:, :], lhs, wkT[:, dt * HD:(dt + 1) * HD],
                        start=(dt == 0), stop=(dt == ND - 1),
                    )
                    nc.tensor.matmul(
                        ps_v[:, :], lhs, wvT[:, dt * HD:(dt + 1) * HD],
                        start=(dt == 0), stop=(dt == ND - 1),
                    )
                # v: cast straight to SBUF (token-major is what PV wants)
                nc.scalar.copy(v_all[:, tt * HD:(tt + 1) * HD], ps_v[:, :])

                # rope coefficient tiles for this token tile, repeated for the
                # 4 q heads: cos4/sin4 [128, 256]
                cos4 = p1.tile([128, 4 * 64], F32, tag="cos4")
                sin4 = p1.tile([128, 4 * 64], F32, tag="sin4")
                for h in range(HL):
                    nc.gpsimd.tensor_copy(
                        cos4[:, h * 64:(h + 1) * 64],
                        cos_all[:, tb * 64:(tb + 1) * 64],
                    )
                    nc.gpsimd.tensor_copy(
                        sin4[:, h * 64:(h + 1) * 64],
                        sin_all[:, tb * 64:(tb + 1) * 64],
                    )
                cos1 = cos_all[:, tb * 64:(tb + 1) * 64]
                sin1 = sin_all[:, tb * 64:(tb + 1) * 64]

                # rope on q (all 4 heads at once): even/odd interleaved pairs
                q_ro = p1.tile([128, OL], BF16, tag="q_ro")
                qv = ps_q.rearrange("p (i two) -> p i two", two=2)
                q_e, q_o = qv[:, :, 0:1].opt(), qv[:, :, 1:2].opt()
                orv = q_ro.rearrange("p (i two) -> p i two", two=2)
                o_e, o_o = orv[:, :, 0:1].opt(), orv[:, :, 1:2].opt()
                ta = p1.tile([128, OL // 2], F32, tag="ta")
                tb_ = p1.tile([128, OL // 2], F32, tag="tb_")
                nc.vector.tensor_mul(ta[:, :], q_e, cos4[:, :])
                nc.vector.tensor_mul(tb_[:, :], q_o, sin4[:, :])
                nc.vector.tensor_sub(o_e, ta[:, :], tb_[:, :])
                nc.vector.tensor_mul(ta[:, :], q_e, sin4[:, :])
                nc.vector.tensor_mul(tb_[:, :], q_o, cos4[:, :])
                nc.vector.tensor_add(o_o, ta[:, :], tb_[:, :])

                # rope on k
                k_ro = p1.tile([128, HD], BF16, tag="k_ro")
                kv_ = ps_k[:, :].rearrange("p (i two) -> p i two", two=2)
                k_e, k_o = kv_[:, :, 0:1].opt(), kv_[:, :, 1:2].opt()
                krv = k_ro.rearrange("p (i two) -> p i two", two=2)
                ko_e, ko_o = krv[:, :, 0:1].opt(), krv[:, :, 1:2].opt()
                tc2 = p1.tile([128, HD // 2], F32, tag="tc2")
                td2 = p1.tile([128, HD // 2], F32, tag="td2")
                nc.vector.tensor_mul(tc2[:, :], k_e, cos1)
                nc.vector.tensor_mul(td2[:, :], k_o, sin1)
                nc.vector.tensor_sub(ko_e, tc2[:, :], td2[:, :])
                nc.vector.tensor_mul(tc2[:, :], k_e, sin1)
                nc.vector.tensor_mul(td2[:, :], k_o, cos1)
                nc.vector.tensor_add(ko_o, tc2[:, :], td2[:, :])

                if dbg is not None and tt == 17:
                    dqro = p1.tile([128, OL], F32, tag="dqro")
                    nc.vector.tensor_copy(dqro[:, :], q_ro[:, :])
                    nc.sync.dma_start(out=dbg["d_qro"][:, :], in_=dqro[:, :])
                    dkro = p1.tile([128, HD], F32, tag="dkro")
                    nc.vector.tensor_copy(dkro[:, :], k_ro[:, :])
                    nc.sync.dma_start(out=dbg["d_kro"][:, :], in_=dkro[:, :])
                    dv = p1.tile([128, HD], F32, tag="dv")
                    nc.vector.tensor_copy(dv[:, :], v_all[:, tt * HD:(tt + 1) * HD])
                    nc.sync.dma_start(out=dbg["d_v"][:, :], in_=dv[:, :])

                # transpose into feature-major q^T / k^T via xbar
                for h in range(HL):
                    nc.sync.dma_start(
                        out=qT_all[:, h * BT + tt * 128: h * BT + (tt + 1) * 128],
                        in_=q_ro[:, h * 128:(h + 1) * 128],
                        transpose=True,
                    )
                nc.sync.dma_start(
                    out=kT_all[:, tt * 128:(tt + 1) * 128],
                    in_=k_ro[:, :],
                    transpose=True,
                )

        if dbg is not None:
            with tc.tile_pool(name="dbg1", bufs=1) as dbp:
                dqT = dbp.tile([128, 128], F32)
                nc.vector.tensor_copy(dqT[:, :], qT_all[:, 2 * BT + 17 * 128: 2 * BT + 18 * 128])
                nc.sync.dma_start(out=dbg["d_qT"][:, :], in_=dqT[:, :])

        # ================= Phase 2: causal attention =================
        with tc.tile_pool(name="p2", bufs=2) as p2, \
             tc.tile_pool(name="p2s", bufs=3) as p2s, \
             tc.tile_pool(name="p2ps", bufs=2, space="PSUM") as p2ps:
            for b in range(B):
                for h in range(HL):
                    for qi in range(NTB):
                        nk = qi + 1                      # causal: k tiles 0..qi
                        nch = (nk + 3) // 4              # 512-wide score chunks
                        p_sb = p2.tile([128, T], BF16, tag="p_sb")
                        sums = p2s.tile([128, 8], F32, tag="sums")
                        qT_lhs = qT_all[
                            :, h * BT + (b * NTB + qi) * 128:
                               h * BT + (b * NTB + qi + 1) * 128]
                        for c in range(nch):
                            w = min(512, nk * 128 - c * 512)
                            ps_s = p2ps.tile([128, 512], F32, tag="ps_s")
                            nc.tensor.matmul(
                                ps_s[:, :w], qT_lhs,
                                kT_all[:, b * T + c * 512: b * T + c * 512 + w],
                                start=True, stop=True,
                            )
                            if c == nch - 1:
                                nc.vector.tensor_add(
                                    ps_s[:, w - 128:w], ps_s[:, w - 128:w],
                                    maskt[:, :],
                                )
                            nc.scalar.activation(
                                p_sb[:, c * 512: c * 512 + w], ps_s[:, :w],
                                AF.Exp, scale=SCALE,
                                accum_out=sums[:, c:c + 1],
                            )
                        rinv = p2s.tile([128, 1], F32, tag="rinv")
                        if nch > 1:
                            ssum = p2s.tile([128, 1], F32, tag="ssum")
                            nc.vector.tensor_reduce(
                                ssum[:, :], sums[:, :nch], AX.X,
                                mybir.AluOpType.add,
                            )
                            nc.vector.reciprocal(rinv[:, :], ssum[:, :])
                        else:
                            nc.vector.reciprocal(rinv[:, :], sums[:, 0:1])

                        # P^T tiles via xbar, then PV (token-major out)
                        pT = p2.tile([128, T], BF16, tag="pT")
                        for tk in range(nk):
                            nc.sync.dma_start(
                                out=pT[:, tk * 128:(tk + 1) * 128],
                                in_=p_sb[:, tk * 128:(tk + 1) * 128],
                                transpose=True,
                            )
                        ps_pv = p2ps.tile([128, HD], F32, tag="ps_pv")
                        for tk in range(nk):
                            nc.tensor.matmul(
                                ps_pv[:, :],
                                pT[:, tk * 128:(tk + 1) * 128],
                                v_all[:, (b * NTB + tk) * HD:
                                         (b * NTB + tk + 1) * HD],
                                start=(tk == 0), stop=(tk == nk - 1),
                            )
                        # normalize by row-sum while casting to bf16
                        attn_sb = p2s.tile([128, HD], BF16, tag="attn_sb")
                        nc.scalar.activation(
                            attn_sb[:, :], ps_pv[:, :], AF.Copy,
                            scale=rinv[:, 0:1],
                        )
                        r0 = (b * NTB + qi) * 128
                        nc.sync.dma_start(
                            out=a2a_in[r0:r0 + 128, h * HD:(h + 1) * HD],
                            in_=attn_sb[:, :],
                        )
                        if dbg is not None and (b, h, qi) == (1, 2, 3):
                            dp = p2.tile([128, 512], F32, tag="dp")
                            nc.vector.tensor_copy(dp[:, :], p_sb[:, :512])
                            nc.sync.dma_start(out=dbg["d_p"][:, :], in_=dp[:, :])
                            nc.sync.dma_start(out=dbg["d_sums"][:, :], in_=sums[:, :])
                            dat = p2.tile([128, HD], F32, tag="dat")
                            nc.vector.tensor_copy(dat[:, :], attn_sb[:, :])
                            nc.sync.dma_start(out=dbg["d_attn"][:, :], in_=dat[:, :])

        # ================= Phase 3: AllToAll + o-proj =================
        nc.gpsimd.collective_compute(
            "AllToAll",
            mybir.AluOpType.bypass,
            replica_groups=[list(range(NCORES))],
            ins=[a2a_in[:, :].opt()],
            outs=[a2a_out[:, :].opt()],
        )

        with tc.tile_pool(name="p3", bufs=2) as p3, \
             tc.tile_pool(name="p3w", bufs=1) as p3w, \
             tc.tile_pool(name="p3ps", bufs=2, space="PSUM") as p3ps:
            NT2 = TOK_SLICE // 128  # 4 token tiles owned by this core
            # attn^T (feature-major lhsT tiles) via transposed DRAM loads:
            # slice (ot, t2) at [:, (ot*NT2+t2)*128 : +128]
            attnT = p3w.tile([128, ND * TOK_SLICE], BF16)
            for ot in range(ND):
                s, cb = ot // 4, ot % 4
                for t2 in range(NT2):
                    nc.scalar.dma_start(
                        out=attnT[:, (ot * NT2 + t2) * 128:
                                     (ot * NT2 + t2 + 1) * 128],
                        in_=a2a_out[s * 512 + t2 * 128: s * 512 + (t2 + 1) * 128,
                                    cb * 128:(cb + 1) * 128],
                        transpose=True,
                    )
            for dc in range(D // 512):  # 8 chunks of 512 output dims
                woT = p3.tile([128, ND * 512], BF16, tag="woT")
                for d4 in range(4):
                    wotmp = p3.tile([128, D], F32, tag="wotmp")
                    nc.sync.dma_start(
                        out=wotmp[:, :],
                        in_=wo[dc * 512 + d4 * 128: dc * 512 + (d4 + 1) * 128, :],
                    )
                    wobf = p3.tile([128, D], BF16, tag="wobf")
                    nc.scalar.copy(wobf[:, :], wotmp[:, :])
                    for ot in range(ND):
                        nc.sync.dma_start(
                            out=woT[:, ot * 512 + d4 * 128: ot * 512 + (d4 + 1) * 128],
                            in_=wobf[:, ot * 128:(ot + 1) * 128],
                            transpose=True,
                        )
                for t2 in range(NT2):
                    ps_o = p3ps.tile([128, 512], F32, tag="ps_o")
                    for ot in range(ND):
                        nc.tensor.matmul(
                            ps_o[:, :],
                            attnT[:, (ot * NT2 + t2) * 128:(ot * NT2 + t2 + 1) * 128],
                            woT[:, ot * 512:(ot + 1) * 512],
                            start=(ot == 0), stop=(ot == ND - 1),
                        )
                    out_sb = p3.tile([128, 512], F32, tag="out_sb")
                    nc.scalar.copy(out_sb[:, :], ps_o[:, :])
                    nc.sync.dma_start(
                        out=out[t2 * 128:(t2 + 1) * 128, dc * 512:(dc + 1) * 512],
                        in_=out_sb[:, :],
                    )


_NC_CACHE = None


def kernel(x, freqs, Wq, Wk, Wv, Wo):
    global _NC_CACHE
    x = np.ascontiguousarray(np.asarray(x, dtype=np.float32)).reshape(BT, D)
    freqs = np.ascontiguousarray(np.asarray(freqs, dtype=np.float32))
    Wq = np.ascontiguousarray(np.asarray(Wq, dtype=np.float32))
    Wk = np.ascontiguousarray(np.asarray(Wk, dtype=np.float32))
    Wv = np.ascontiguousarray(np.asarray(Wv, dtype=np.float32))
    Wo = np.ascontiguousarray(np.asarray(Wo, dtype=np.float32))

    if _NC_CACHE is None:
        _NC_CACHE = _build()
    nc = _NC_CACHE

    in_maps = []
    for c in range(NCORES):
        in_maps.append({
            "x": x,
            "freqs": freqs,
            "wq": np.ascontiguousarray(Wq[c * OL:(c + 1) * OL, :]),
            "wk": np.ascontiguousarray(Wk[c * HD:(c + 1) * HD, :]),
            "wv": np.ascontiguousarray(Wv[c * HD:(c + 1) * HD, :]),
            "wo": Wo,
        })
    trace = os.environ.get("KERNEL_TRACE") == "1"
    res = run_bass_kernel_spmd(
        nc, in_maps, core_ids=list(range(NCORES)), trace=trace,
    )
    if trace and res.exec_time_ns is not None:
        print(f"HW exec time: {res.exec_time_ns} ns")
        if res.instructions_and_trace is not None:
            print(f"trace: {res.instructions_and_trace}")
    out = np.concatenate(
        [res.results[c]["out"] for c in range(NCORES)], axis=0
    ).reshape(B, T, D)
    return out


# revision 10
# speedup vs baseline: 2.7715x; 2.7715x over previous
"""GroupedQueryAttention forward, 8-way tensor-parallel over heads on TRN2.

Problem: B=2, T=2048, D=4096, H=32 q-heads, KV=8 kv-heads, HD=128, causal,
interleaved RoPE, softmax, o-proj.

Sharding (8 NeuronCores):
  - core c owns q-heads 4c..4c+3 (Wq rows c*512..) and kv-head c (Wk/Wv rows
    c*128..). x and Wo are replicated.
  - Each core computes q/k/v proj + RoPE + causal attention for its heads over
    ALL tokens, writes attn-out (token-major, bf16) into an AllToAll buffer
    sharded by token-block.
  - AllToAll redistributes: core c ends with the full 4096-dim attn-out for
    tokens c*512..(c+1)*512, computes the o-proj for those tokens, and writes
    its [512, 4096] slice of the output. Host concatenates the 8 slices.

All matmuls run in bf16 (fp32 PSUM accumulation). fp32 operands are cast to
bf16 on-chip. K-first (transposed) layouts come from the HWDGE xbar
DMA-transpose, batched into wide calls with contiguous 3D outputs (the xbar
transposes each 128-col block of the source independently: out[p, c, j] =
in[j, c*128+p]). Large fp32 loads ride the SWDGE (gpsimd) queues to keep the
two HWDGE rings free for transposes.
"""

import math
import os

import numpy as np

import concourse.bass as bass
import concourse.mybir as mybir
import concourse.tile as tile
from concourse import bacc
from concourse.bass_utils import run_bass_kernel_spmd
from concourse.masks import make_causal_mask

F32 = mybir.dt.float32
BF16 = mybir.dt.bfloat16
AF = mybir.ActivationFunctionType
AX = mybir.AxisListType

B, T, D = 2, 2048, 4096
H, KV, HD = 32, 8, 128
NCORES = 8
BT = B * T                  # 4096 flattened tokens
HL = H // NCORES            # 4 q heads per core
OL = HL * HD                # 512 local o-dim
NT = BT // 128              # 32 token tiles
NTB = T // 128              # 16 token tiles per batch
ND = D // 128               # 32 d k-tiles
TOK_SLICE = BT // NCORES    # 512 tokens owned per core for o-proj
SCALE = 1.0 / math.sqrt(HD)
MASK_VAL = -1e9


def _build():
    nc = bacc.Bacc(
        "TRN2",
        target_bir_lowering=False,
        debug=False,
        enable_asserts=False,
        num_devices=NCORES,
    )
    x = nc.dram_tensor("x", [BT, D], F32, kind="ExternalInput").ap()
    freqs = nc.dram_tensor("freqs", [T, HD // 2, 2], F32, kind="ExternalInput").ap()
    wq = nc.dram_tensor("wq", [OL, D], F32, kind="ExternalInput").ap()
    wk = nc.dram_tensor("wk", [HD, D], F32, kind="ExternalInput").ap()
    wv = nc.dram_tensor("wv", [HD, D], F32, kind="ExternalInput").ap()
    wo = nc.dram_tensor("wo", [D, D], F32, kind="ExternalInput").ap()
    out = nc.dram_tensor("out", [TOK_SLICE, D], F32, kind="ExternalOutput").ap()

    with tile.TileContext(nc) as tc:
        _body(tc, nc, x, freqs, wq, wk, wv, wo, out)
    nc.compile()
    return nc


def _body(tc, nc, x, freqs, wq, wk, wv, wo, out):
    with tc.tile_pool(name="persist", bufs=1) as persist, \
         tc.tile_pool(name="dram", bufs=1, space="DRAM") as dram:
        # q^T, (tt, h)-major: col of (tile tt, head h) = (tt*HL + h)*128
        qT_all = persist.tile([128, NT * HL * 128], BF16)
        # k^T: [hd, 4096 tokens]
        kT_all = persist.tile([128, BT], BF16)
        # v token-major: [token-part, tile * 128hd]
        v_all = persist.tile([128, NT * HD], BF16)
        maskt = persist.tile([128, 128], F32)
        make_causal_mask(nc, maskt[:, :], mask_val=MASK_VAL)
        cos_all = persist.tile([128, NTB * 64], F32)
        sin_all = persist.tile([128, NTB * 64], F32)
        for tt in range(NTB):
            nc.sync.dma_start(
                out=cos_all[:, tt * 64:(tt + 1) * 64],
                in_=freqs[tt * 128:(tt + 1) * 128, :, 0:1].opt(),
            )
            nc.sync.dma_start(
                out=sin_all[:, tt * 64:(tt + 1) * 64],
                in_=freqs[tt * 128:(tt + 1) * 128, :, 1:2].opt(),
            )

        a2a_in = dram.tile([BT, OL], BF16)
        a2a_out = dram.tile([BT, OL], BF16)

        # ================= Phase 1: qkv proj + rope =================
        with tc.tile_pool(name="p1", bufs=2) as p1, \
             tc.tile_pool(name="p1w", bufs=1) as p1w, \
             tc.tile_pool(name="p1ps", bufs=2, space="PSUM") as p1ps:
            # wqT (ob, dt)-major: col block (ob, dt) at (ob*ND + dt)*128.
            # matmul rhs for dt = 3D AP [128, ob:4 (stride ND*128), 128].
            wqT = p1w.tile([128, HL * ND * 128], BF16)
            wqT4 = wqT.rearrange("p (ob dt j) -> p ob dt j", ob=HL, dt=ND)
            wkT = p1w.tile([128, ND * HD], BF16)
            wvT = p1w.tile([128, ND * HD], BF16)
            for ob in range(HL):
                wtmp = p1.tile([128, D], F32, tag="ftmp")
                nc.gpsimd.dma_start(out=wtmp[:, :], in_=wq[ob * 128:(ob + 1) * 128, :])
                wbf = p1.tile([128, D], BF16, tag="btmp")
                nc.scalar.copy(wbf[:, :], wtmp[:, :])
                nc.sync.dma_start(
                    out=wqT.rearrange("p (b j) -> p b j", j=128)[:, ob * ND:(ob + 1) * ND, :],
                    in_=wbf[:, :], transpose=True,
                )
            for w_in, w_t in ((wk, wkT), (wv, wvT)):
                wtmp = p1.tile([128, D], F32, tag="ftmp")
                nc.gpsimd.dma_start(out=wtmp[:, :], in_=w_in[:, :])
                wbf = p1.tile([128, D], BF16, tag="btmp")
                nc.scalar.copy(wbf[:, :], wtmp[:, :])
                nc.sync.dma_start(
                    out=w_t.rearrange("p (b j) -> p b j", j=128),
                    in_=wbf[:, :], transpose=True,
                )

            for tt in range(NT):
                tb = tt % NTB
                xtmp = p1.tile([128, D], F32, tag="ftmp")
                nc.gpsimd.dma_start(out=xtmp[:, :], in_=x[tt * 128:(tt + 1) * 128, :])
                xbf = p1.tile([128, D], BF16, tag="btmp")
                nc.scalar.copy(xbf[:, :], xtmp[:, :])
                xT = p1.tile([128, D], BF16, tag="xT")
                nc.sync.dma_start(
                    out=xT.rearrange("p (b j) -> p b j", j=128),
                    in_=xbf[:, :], transpose=True,
                )
                ps_q = p1ps.tile([128, OL], F32, tag="ps_q")
                ps_k = p1ps.tile([128, HD], F32, tag="ps_k")
                ps_v = p1ps.tile([128, HD], F32, tag="ps_v")
                for dt in range(ND):
                    lhs = xT[:, dt * 128:(dt + 1) * 128]
                    nc.tensor.matmul(
                        ps_q[:, :], lhs, wqT4[:, :, dt, :].opt(),
                        start=(dt == 0), stop=(dt == ND - 1),
                    )
                    nc.tensor.matmul(
                        ps_k[:, :], lhs, wkT[:, dt * HD:(dt + 1) * HD],
                        start=(dt == 0), stop=(dt == ND - 1),
                    )
                    nc.tensor.matmul(
                        ps_v[:, :], lhs, wvT[:, dt * HD:(dt + 1) * HD],
                        start=(dt == 0), stop=(dt == ND - 1),
                    )
                nc.scalar.copy(v_all[:, tt * HD:(tt + 1) * HD], ps_v[:, :])

                cos4 = p1.tile([128, 4 * 64], F32, tag="cos4")
                sin4 = p1.tile([128, 4 * 64], F32, tag="sin4")
                for h in range(HL):
                    nc.gpsimd.tensor_copy(
                        cos4[:, h * 64:(h + 1) * 64],
                        cos_all[:, tb * 64:(tb + 1) * 64],
                    )
                    nc.gpsimd.tensor_copy(
                        sin4[:, h * 64:(h + 1) * 64],
                        sin_all[:, tb * 64:(tb + 1) * 64],
                    )
                cos1 = cos_all[:, tb * 64:(tb + 1) * 64]
                sin1 = sin_all[:, tb * 64:(tb + 1) * 64]

                q_ro = p1.tile([128, OL], BF16, tag="q_ro")
                qv = ps_q.rearrange("p (i two) -> p i two", two=2)
                q_e, q_o = qv[:, :, 0:1].opt(), qv[:, :, 1:2].opt()
                orv = q_ro.rearrange("p (i two) -> p i two", two=2)
                o_e, o_o = orv[:, :, 0:1].opt(), orv[:, :, 1:2].opt()
                ta = p1.tile([128, OL // 2], F32, tag="ta")
                tb_ = p1.tile([128, OL // 2], F32, tag="tb_")
                nc.vector.tensor_mul(ta[:, :], q_e, cos4[:, :])
                nc.vector.tensor_mul(tb_[:, :], q_o, sin4[:, :])
                nc.vector.tensor_sub(o_e, ta[:, :], tb_[:, :])
                nc.vector.tensor_mul(ta[:, :], q_e, sin4[:, :])
                nc.vector.tensor_mul(tb_[:, :], q_o, cos4[:, :])
                nc.vector.tensor_add(o_o, ta[:, :], tb_[:, :])

                k_ro = p1.tile([128, HD], BF16, tag="k_ro")
                kv_ = ps_k.rearrange("p (i two) -> p i two", two=2)
                k_e, k_o = kv_[:, :, 0:1].opt(), kv_[:, :, 1:2].opt()
                krv = k_ro.rearrange("p (i two) -> p i two", two=2)
                ko_e, ko_o = krv[:, :, 0:1].opt(), krv[:, :, 1:2].opt()
                tc2 = p1.tile([128, HD // 2], F32, tag="tc2")
                td2 = p1.tile([128, HD // 2], F32, tag="td2")
                nc.vector.tensor_mul(tc2[:, :], k_e, cos1)
                nc.vector.tensor_mul(td2[:, :], k_o, sin1)
                nc.vector.tensor_sub(ko_e, tc2[:, :], td2[:, :])
                nc.vector.tensor_mul(tc2[:, :], k_e, sin1)
                nc.vector.tensor_mul(td2[:, :], k_o, cos1)
                nc.vector.tensor_add(ko_o, tc2[:, :], td2[:, :])

                # q^T for 4 heads in one wide xbar into (tt, h)-major storage
                nc.scalar.dma_start(
                    out=qT_all.rearrange("p (b j) -> p b j", j=128)[
                        :, tt * HL:(tt + 1) * HL, :],
                    in_=q_ro[:, :], transpose=True,
                )
                nc.scalar.dma_start(
                    out=kT_all[:, tt * 128:(tt + 1) * 128],
                    in_=k_ro[:, :], transpose=True,
                )

        # ================= Phase 2: causal attention =================
        with tc.tile_pool(name="p2", bufs=2) as p2, \
             tc.tile_pool(name="p2s", bufs=3) as p2s, \
             tc.tile_pool(name="p2ps", bufs=2, space="PSUM") as p2ps:
            for b in range(B):
                for qi in range(NTB):
                    attn_sb = p2s.tile([128, OL], BF16, tag="attn_sb")
                    for h in range(HL):
                        nk = qi + 1
                        nch = (nk + 3) // 4
                        p_sb = p2.tile([128, T], BF16, tag="p_sb")
                        sums = p2s.tile([128, 8], F32, tag="sums")
                        qT_lhs = qT_all[
                            :, ((b * NTB + qi) * HL + h) * 128:
                               ((b * NTB + qi) * HL + h + 1) * 128]
                        for c in range(nch):
                            w = min(512, nk * 128 - c * 512)
                            ps_s = p2ps.tile([128, 512], F32, tag="ps_s")
                            nc.tensor.matmul(
                                ps_s[:, :w], qT_lhs,
                                kT_all[:, b * T + c * 512: b * T + c * 512 + w],
                                start=True, stop=True,
                            )
                            if c == nch - 1:
                                nc.vector.tensor_add(
                                    ps_s[:, w - 128:w], ps_s[:, w - 128:w],
                                    maskt[:, :],
                                )
                            nc.scalar.activation(
                                p_sb[:, c * 512: c * 512 + w], ps_s[:, :w],
                                AF.Exp, scale=SCALE,
                                accum_out=sums[:, c:c + 1],
                            )
                        rinv = p2s.tile([128, 1], F32, tag="rinv")
                        if nch > 1:
                            ssum = p2s.tile([128, 1], F32, tag="ssum")
                            nc.vector.tensor_reduce(
                                ssum[:, :], sums[:, :nch], AX.X,
                                mybir.AluOpType.add,
                            )
                            nc.vector.reciprocal(rinv[:, :], ssum[:, :])
                        else:
                            nc.vector.reciprocal(rinv[:, :], sums[:, 0:1])

                        # P^T via ONE wide xbar (contiguous tk-blocks)
                        pT = p2.tile([128, T], BF16, tag="pT")
                        nc.sync.dma_start(
                            out=pT.rearrange("p (b j) -> p b j", j=128)[:, :nk, :],
                            in_=p_sb[:, :nk * 128], transpose=True,
                        )
                        ps_pv = p2ps.tile([128, HD], F32, tag="ps_pv")
                        for tk in range(nk):
                            nc.tensor.matmul(
                                ps_pv[:, :],
                                pT[:, tk * 128:(tk + 1) * 128],
                                v_all[:, (b * NTB + tk) * HD:
                                         (b * NTB + tk + 1) * HD],
                                start=(tk == 0), stop=(tk == nk - 1),
                            )
                        # normalize by row-sum while casting to bf16; gather
                        # the 4 heads of this (b, qi) into one staging tile
                        nc.scalar.activation(
                            attn_sb[:, h * HD:(h + 1) * HD], ps_pv[:, :],
                            AF.Copy, scale=rinv[:, 0:1],
                        )
                        if h == HL - 1:
                            r0 = (b * NTB + qi) * 128
                            nc.sync.dma_start(
                                out=a2a_in[r0:r0 + 128, :],
                                in_=attn_sb[:, :],
                            )

        # ================= Phase 3: AllToAll + o-proj =================
        nc.gpsimd.collective_compute(
            "AllToAll",
            mybir.AluOpType.bypass,
            replica_groups=[list(range(NCORES))],
            ins=[a2a_in[:, :].opt()],
            outs=[a2a_out[:, :].opt()],
        )

        with tc.tile_pool(name="p3", bufs=2) as p3, \
             tc.tile_pool(name="p3w", bufs=1) as p3w, \
             tc.tile_pool(name="p3ps", bufs=2, space="PSUM") as p3ps:
            NT2 = TOK_SLICE // 128  # 4 token tiles owned by this core
            # attn^T, (s, t2, cb)-major: lhsT for (ot=s*4+cb, t2) at
            # col ((s*NT2 + t2)*4 + cb) * 128
            attnT = p3w.tile([128, ND * TOK_SLICE], BF16)
            for s in range(NCORES):
                for t2 in range(NT2):
                    nc.scalar.dma_start(
                        out=attnT.rearrange("p (b j) -> p b j", j=128)[
                            :, (s * NT2 + t2) * 4:(s * NT2 + t2 + 1) * 4, :],
                        in_=a2a_out[s * 512 + t2 * 128: s * 512 + (t2 + 1) * 128, :],
                        transpose=True,
                    )
            # woT (d4, ot)-major: col block (d4, ot) at (d4*ND + ot)*128;
            # matmul rhs for ot = 3D AP [128, d4:4 (stride ND*128), 128]
            for dc in range(D // 512):
                woT = p3.tile([128, 4 * ND * 128], BF16, tag="woT")
                woT4 = woT.rearrange("p (d4 ot j) -> p d4 ot j", d4=4, ot=ND)
                for d4 in range(4):
                    wotmp = p3.tile([128, D], F32, tag="wotmp")
                    nc.gpsimd.dma_start(
                        out=wotmp[:, :],
                        in_=wo[dc * 512 + d4 * 128: dc * 512 + (d4 + 1) * 128, :],
                    )
                    wobf = p3.tile([128, D], BF16, tag="wobf")
                    nc.scalar.copy(wobf[:, :], wotmp[:, :])
                    nc.sync.dma_start(
                        out=woT.rearrange("p (b j) -> p b j", j=128)[
                            :, d4 * ND:(d4 + 1) * ND, :],
                        in_=wobf[:, :], transpose=True,
                    )
                for t2 in range(NT2):
                    ps_o = p3ps.tile([128, 512], F32, tag="ps_o")
                    for ot in range(ND):
                        s, cb = ot // 4, ot % 4
                        nc.tensor.matmul(
                            ps_o[:, :],
                            attnT[:, ((s * NT2 + t2) * 4 + cb) * 128:
                                     ((s * NT2 + t2) * 4 + cb + 1) * 128],
                            woT4[:, :, ot, :].opt(),
                            start=(ot == 0), stop=(ot == ND - 1),
                        )
                    out_sb = p3.tile([128, 512], F32, tag="out_sb")
                    nc.scalar.copy(out_sb[:, :], ps_o[:, :])
                    nc.sync.dma_start(
                        out=out[t2 * 128:(t2 + 1) * 128, dc * 512:(dc + 1) * 512],
                        in_=out_sb[:, :],
                    )


_NC_CACHE = None


def kernel(x, freqs, Wq, Wk, Wv, Wo):
    global _NC_CACHE
    x = np.ascontiguousarray(np.asarray(x, dtype=np.float32)).reshape(BT, D)
    freqs = np.ascontiguousarray(np.asarray(freqs, dtype=np.float32))
    Wq = np.ascontiguousarray(np.asarray(Wq, dtype=np.float32))
    Wk = np.ascontiguousarray(np.asarray(Wk, dtype=np.float32))
    Wv = np.ascontiguousarray(np.asarray(Wv, dtype=np.float32))
    Wo = np.ascontiguousarray(np.asarray(Wo, dtype=np.float32))

    if _NC_CACHE is None:
        _NC_CACHE = _build()
    nc = _NC_CACHE

    in_maps = []
    for c in range(NCORES):
        in_maps.append({
            "x": x,
            "freqs": freqs,
            "wq": np.ascontiguousarray(Wq[c * OL:(c + 1) * OL, :]),
            "wk": np.ascontiguousarray(Wk[c * HD:(c + 1) * HD, :]),
            "wv": np.ascontiguousarray(Wv[c * HD:(c + 1) * HD, :]),
            "wo": Wo,
        })
    trace = os.environ.get("KERNEL_TRACE") == "1"
    res = run_bass_kernel_spmd(
        nc, in_maps, core_ids=list(range(NCORES)), trace=trace,
    )
    if trace and res.exec_time_ns is not None:
        print(f"HW exec time: {res.exec_time_ns} ns")
        if res.instructions_and_trace is not None:
            _analyze(res.instructions_and_trace[0])
    out = np.concatenate(
        [res.results[c]["out"] for c in range(NCORES)], axis=0
    ).reshape(B, T, D)
    return out


def _analyze(insts):
    """Per-engine busy + top source-line aggregation from annotated insts."""
    from collections import defaultdict
    eng_busy = defaultdict(int)
    eng_n = defaultdict(int)
    line_busy = defaultdict(int)
    line_n = defaultdict(int)
    t0 = min(i.timestamp for i in insts)
    t1 = max(i.timestamp + i.duration for i in insts)
    for i in insts:
        e = str(i.engine)
        eng_busy[e] += i.duration
        eng_n[e] += 1
        key = (e.split(".")[-1], i.source_line, (i.name or "")[:18])
        line_busy[key] += i.duration
        line_n[key] += 1
    span = t1 - t0
    print(f"== span {span/1e3:.1f} us ==")
    for e, b in sorted(eng_busy.items(), key=lambda kv: -kv[1]):
        print(f"  {e:28s} busy={b/1e3:9.1f}us ({100*b/span:5.1f}%) n={eng_n[e]}")
    print("== top lines ==")
    for k, b in sorted(line_busy.items(), key=lambda kv: -kv[1])[:24]:
        print(f"  {k[0][:10]:10s} L{str(k[1]):>5s} {k[2]:18s} busy={b/1e3:9.1f}us n={line_n[k]}")
